# revision 1
# baseline (speedup 1.0000x reference)
"""Trainium2 Bass kernel for nn_Hankel (MPS chain over encoded trajectory).

Math (per sample b):
  h   = relu(x @ W1.T + b1)            [T, HID]
  enc = relu(h @ W2.T + b2)            [T, ENC]
  v0  = enc[0] @ H_first[0]            [R]
  for t in 0..T-3:  M_t = einsum('e,per->pr', enc[t+1], H_mid[t]); v = v @ M_t
  out = v @ (enc[T-1] @ H_last[:,:,0].T)   scalar

Strategy: pure data parallel over 8 cores (1024 samples each).
Per core, per 128-sample tile:
  - encoder as PE matmuls (weights stationary), ACT relu+bias on PSUM evac
  - M_t formed as matmul: stationary enc_t^T [e=128, b=128] tile,
    moving H_mid[t] host-permuted to [e, (r p)]; out PSUM [b, (r p)]
  - per-sample p-contraction v' = sum_p v[p] M[(r p)] on DVE:
    fp16 broadcast-multiply + log2 tree reduction along p
  - final dot via tensor_tensor_reduce into fp32
"""

import sys

for _p in ("/opt/trn_rl_repo", "/root/.axon_site/_ro/trn_rl_repo"):
    if _p not in sys.path:
        sys.path.append(_p)

import numpy as np
import ml_dtypes

B, T, D, HID, ENC, R = 8192, 12, 64, 512, 128, 64
NCORES = 8
BC = B // NCORES          # samples per core
NTILES = BC // 128        # 8 tiles of 128 samples
BT = BC * T               # 12288 (t-major: col = t*BC + b)
NCHUNK = BT // 512        # 24 encoder n-chunks
F16NP = np.float16
# The MPS chain decays ~80x per step; rescale H tensors by 2^6 (exact in fp)
# so fp16 intermediates stay in range, and unscale the output on host.
SCALE = 64.0
NSCALED = 12              # Hf + 10*Hm + Hl each carry one factor of SCALE

_CACHE = {}


def _build():
    import concourse.bass as bass
    import concourse.tile as tile
    from concourse import bacc, mybir
    from contextlib import ExitStack

    F16 = mybir.dt.float16
    F32 = mybir.dt.float32
    AX = mybir.AxisListType
    OP = mybir.AluOpType
    AF = mybir.ActivationFunctionType

    nc = bacc.Bacc(None, target_bir_lowering=False, debug=False)

    # xT carries a trailing ones-row (row D) so biases fold into matmuls
    xT = nc.declare_dram_parameter("xT", [D + 1, BT], F16, isOutput=False)
    w1 = nc.declare_dram_parameter("w1", [D + 1, HID], F16, isOutput=False)
    w2 = nc.declare_dram_parameter("w2", [128, HID // 128, ENC], F16, isOutput=False)
    b2 = nc.declare_dram_parameter("b2", [1, ENC], F16, isOutput=False)
    hm = nc.declare_dram_parameter("hm", [T - 2, ENC, R * R], F16, isOutput=False)
    hf = nc.declare_dram_parameter("hf", [ENC, R], F16, isOutput=False)
    hl = nc.declare_dram_parameter("hl", [ENC, R], F16, isOutput=False)
    out = nc.declare_dram_parameter("out", [128, NTILES], F32, isOutput=True)

    with tile.TileContext(nc) as tc, ExitStack() as ctx:
        const = ctx.enter_context(tc.tile_pool(name="const", bufs=1))
        encp = ctx.enter_context(tc.tile_pool(name="encp", bufs=1))
        hbuf = ctx.enter_context(tc.tile_pool(name="hbuf", bufs=2))
        hwork = ctx.enter_context(tc.tile_pool(name="hwork", bufs=2))
        mwork = ctx.enter_context(tc.tile_pool(name="mwork", bufs=2))
        twork = ctx.enter_context(tc.tile_pool(name="twork", bufs=2))
        ps_a = ctx.enter_context(tc.tile_pool(name="ps_a", bufs=1, space="PSUM"))
        ps_b = ctx.enter_context(tc.tile_pool(name="ps_b", bufs=1, space="PSUM"))
        ps_mm = ctx.enter_context(tc.tile_pool(name="ps_mm", bufs=2, space="PSUM"))

        # ---- load constants / inputs to SBUF ----
        xT_sb = const.tile([D + 1, BT], F16)
        nc.sync.dma_start(out=xT_sb[:], in_=xT[:])
        w1_sb = const.tile([D + 1, HID], F16)
        nc.sync.dma_start(out=w1_sb[:], in_=w1[:])
        w2_sb = const.tile([128, HID // 128, ENC], F16)
        nc.sync.dma_start(out=w2_sb[:], in_=w2[:])
        b2_sb = const.tile([1, ENC], F16)
        nc.sync.dma_start(out=b2_sb[:], in_=b2[:])
        hf_sb = const.tile([ENC, R], F16)
        nc.sync.dma_start(out=hf_sb[:], in_=hf[:])
        hl_sb = const.tile([ENC, R], F16)
        nc.sync.dma_start(out=hl_sb[:], in_=hl[:])
        ones_sb = const.tile([1, 512], F16)
        nc.vector.memset(ones_sb[:], 1.0)

        encT_sb = const.tile([ENC, BT], F16)   # [e, t*BC + b]
        v_sb = const.tile([128, NTILES, R], F16)
        out_sb = const.tile([128, NTILES], F32)

        # ---- encoder: 24 chunks of 512 bt-columns ----
        NH = HID // 128  # 4 hid chunks
        for n in range(NCHUNK):
            ncol = slice(n * 512, (n + 1) * 512)
            h_sb = hwork.tile([128, NH, 512], F16, tag="h_sb")
            for cpair in range(NH // 2):
                ps1 = ps_a.tile([128, 2, 512], F32, tag="ps1")
                for ci in range(2):
                    c = cpair * 2 + ci
                    nc.tensor.matmul(
                        ps1[:, ci, :],
                        w1_sb[:, c * 128:(c + 1) * 128],
                        xT_sb[:, ncol],
                    )
                for ci in range(2):
                    c = cpair * 2 + ci
                    nc.scalar.activation(h_sb[:, c, :], ps1[:, ci, :], AF.Relu)
            ps2 = ps_b.tile([128, 512], F32, tag="ps2")
            for c in range(NH):
                nc.tensor.matmul(
                    ps2[:],
                    w2_sb[:, c, :],
                    h_sb[:, c, :],
                    start=(c == 0),
                    stop=False,
                )
            # bias row: K=1 matmul against the ones-row of xT
            nc.tensor.matmul(
                ps2[:], b2_sb[:], ones_sb[:], start=False, stop=True,
            )
            nc.scalar.activation(encT_sb[:, ncol], ps2[:], AF.Relu)

        # ---- v0 = enc_0 @ H_first ----
        for it in range(NTILES):
            bcol = slice(it * 128, (it + 1) * 128)  # t=0 block
            psv = ps_b.tile([128, R], F32, tag="ps2")
            nc.tensor.matmul(psv[:], encT_sb[:, bcol], hf_sb[:])
            nc.scalar.activation(v_sb[:, it, :], psv[:], AF.Copy)

        # ---- chain: t = 0..9 ----
        for t in range(T - 2):
            h_t = hbuf.tile([ENC, R * R], F16, tag="h_t")
            nc.sync.dma_start(out=h_t[:], in_=hm[t])
            for it in range(NTILES):
                bcol = slice((t + 1) * BC + it * 128, (t + 1) * BC + (it + 1) * 128)
                m_sb = mwork.tile([128, R * R], F16, tag="m_sb")
                for j in range(4):
                    psm = ps_mm.tile([128, 1024], F32, tag="psm")
                    for jj in range(2):
                        nj = j * 1024 + jj * 512
                        nc.tensor.matmul(
                            psm[:, jj * 512:(jj + 1) * 512],
                            encT_sb[:, bcol],
                            h_t[:, nj:nj + 512],
                        )
                    nc.scalar.activation(
                        m_sb[:, j * 1024:(j + 1) * 1024], psm[:], AF.Copy,
                    )
                # v'[b, r] = sum_p M[b, (r p)] * v[b, p]
                m3 = m_sb[:].rearrange("b (r p) -> b r p", p=R)
                tmp = twork.tile([128, R, R], F16, tag="tmp")
                vbc = v_sb[:, it, :].unsqueeze(1).broadcast_to([128, R, R])
                nc.vector.tensor_tensor(out=tmp[:], in0=m3, in1=vbc, op=OP.mult)
                w = R // 2
                src = tmp
                while w >= 2:
                    dst = twork.tile([128, R, w], F16, tag=f"tr{w}")
                    nc.vector.tensor_tensor(
                        out=dst[:], in0=src[:, :, 0:w], in1=src[:, :, w:2 * w],
                        op=OP.add,
                    )
                    src = dst
                    w //= 2
                nc.vector.tensor_tensor(
                    out=v_sb[:, it, :].unsqueeze(2),
                    in0=src[:, :, 0:1], in1=src[:, :, 1:2], op=OP.add,
                )

        # ---- last: dot(v, enc_{T-1} @ H_last) ----
        for it in range(NTILES):
            bcol = slice((T - 1) * BC + it * 128, (T - 1) * BC + (it + 1) * 128)
            psl = ps_b.tile([128, R], F32, tag="ps2")
            nc.tensor.matmul(psl[:], encT_sb[:, bcol], hl_sb[:])
            last_sb = hwork.tile([128, R], F16, tag="last_sb")
            nc.scalar.activation(last_sb[:], psl[:], AF.Copy)
            prod = hwork.tile([128, R], F32, tag="prod")
            nc.vector.tensor_tensor(
                out=prod[:], in0=last_sb[:], in1=v_sb[:, it, :], op=OP.mult
            )
            nc.vector.tensor_reduce(
                out_sb[:, it:it + 1], prod[:], axis=AX.X, op=OP.add
            )

        nc.sync.dma_start(out=out[:], in_=out_sb[:])

    nc.compile()
    return nc


def _prep_inputs(x, W1, b1, W2, b2, H_first, H_mid, H_last):
    """Host-side prep: shard x, transpose/permute/cast weights."""
    ins = []
    # w1 augmented with bias row (pairs with the ones-row of xT)
    w1h = np.concatenate([W1.T, b1[None, :]], axis=0).astype(F16NP)  # [D+1, HID]
    # w2 pre-chunked: w2h[p, c, e] = W2[e, c*128 + p]
    w2h = np.ascontiguousarray(
        W2.T.reshape(HID // 128, 128, ENC).transpose(1, 0, 2)
    ).astype(F16NP)
    b2h = b2[None, :].astype(F16NP)                # [1, ENC]
    # H_mid[t, p, e, r] -> hm[t, e, (r p)] : hm[t,e,r,p] = H_mid[t,p,e,r]
    hmh = (np.ascontiguousarray(np.transpose(H_mid, (0, 2, 3, 1))).reshape(
        T - 2, ENC, R * R
    ) * SCALE).astype(F16NP)
    hfh = (H_first[0] * SCALE).astype(F16NP)       # [ENC, R]
    hlh = (np.ascontiguousarray(H_last[:, :, 0].T) * SCALE).astype(F16NP)
    for c in range(NCORES):
        xs = x[c * BC:(c + 1) * BC]                # [BC, T, D]
        # xT[d, t*BC + b] = x[b, t, d]; trailing ones row
        xTh = np.empty((D + 1, BT), dtype=F16NP)
        xTh[:D] = np.transpose(xs, (2, 1, 0)).reshape(D, BT)
        xTh[D] = 1.0
        ins.append({
            "xT": xTh, "w1": w1h, "w2": w2h, "b2": b2h,
            "hm": hmh, "hf": hfh, "hl": hlh,
        })
    return ins


def kernel(x, W1, b1, W2, b2, H_first, H_mid, H_last):
    from concourse.bass_utils import run_bass_kernel_spmd

    if "nc" not in _CACHE:
        _CACHE["nc"] = _build()
    nc = _CACHE["nc"]

    in_maps = _prep_inputs(x, W1, b1, W2, b2, H_first, H_mid, H_last)
    res = run_bass_kernel_spmd(nc, in_maps, core_ids=list(range(NCORES)))
    # out[b_in_tile, tile] per core -> flat [BC] with index tile*128 + b
    outs = [
        np.asarray(res.results[c]["out"]).T.reshape(BC) for c in range(NCORES)
    ]
    full = np.concatenate(outs, axis=0).astype(np.float64)
    return (full / SCALE**NSCALED).astype(np.float32)

